# revision 30
# baseline (speedup 1.0000x reference)
"""GPT-J attention (B=1, S=2048, D=4096, H=16, HD=256, rot=64) on 8 TRN2 cores.

Strategy: tensor-parallel over heads (2 heads/core) for QKV+attention, then
column-parallel out-projection: after attention, each core AllGathers the full
ctx^T (its [512, 512] per query group -> shared [4096, 512]) and computes a
complete 512-column slice of the output (contracting the full d=4096), so no
ReduceScatter of 16.8MB partials is needed -- the host just concatenates the
8 column shards. Collective volume drops 8x and the reduction disappears.

Phases are interleaved so the AllGathers hide under PE work and the kernel
ends on matmuls, not comms:
  P0 (QKV for s<1024) -> A0,A1 (+AG0,AG1) -> O0,O1 -> P1 -> A2,A3 -> O2,O3

Host-side prep (cheap, numpy):
  - hsT = hidden_states.T so all device matmuls contract over the partition dim
  - per-core weight shards pre-transposed; Wq/Wk rows permuted within each head
    (even rot dims, odd rot dims, rest) so rotary becomes plain block ops
  - woT2 = Wo[cols of this core, :].T  (column shard of the out-projection)
  - 1/sqrt(HD) folded into Wq; sin/cos tables and causal mask tiles precomputed

Matmul operands are bf16 (fp32r measured 2 cyc/row on HW; bf16 is 1), all
accumulation in fp32 PSUM; softmax exp input, masks, rotary and the softmax
normalization run in fp32. q/K^T/V stay SBUF-resident between phases; ctx^T
roundtrips through DRAM via the AllGather. PSUM evictions alternate DVE/ACT.
"""

import contextlib
import numpy as np
import ml_dtypes

import concourse.bass as bass
import concourse.tile as tile
import concourse.mybir as mybir
from concourse import bacc
from concourse.bass_utils import run_bass_kernel_spmd

B, S, D = 1, 2048, 4096
H, HD, ROT = 16, 256, 64
NCORES = 8
HL = H // NCORES          # heads per core = 2
EL = D // NCORES          # local e width = 512
HALFW = S // 2            # 1024
P = 128
NROT2 = ROT // 2          # 32

f32 = mybir.dt.float32
bf16 = mybir.dt.bfloat16
EXP = mybir.ActivationFunctionType.Exp
COPY = mybir.ActivationFunctionType.Copy
ADD = mybir.AluOpType.add
MUL = mybir.AluOpType.mult
SUB = mybir.AluOpType.subtract

_CACHE = {}


def _emit(nc, t):
    """Emit the whole per-core program inside a TileContext."""
    with tile.TileContext(nc) as tc:
        with contextlib.ExitStack() as _stk:
            ec = _stk.enter_context
            const_pool = ec(tc.tile_pool(name="const", bufs=1))
            wpan_pool = ec(tc.tile_pool(name="wpan", bufs=4))
            stage_pool = ec(tc.tile_pool(name="stage", bufs=4))
            hst_pool = ec(tc.tile_pool(name="hst", bufs=32))
            qres_pool = ec(tc.tile_pool(name="qres", bufs=16))
            kres_pool = ec(tc.tile_pool(name="kres", bufs=16))
            vres_pool = ec(tc.tile_pool(name="vres", bufs=16))
            wot_pool = ec(tc.tile_pool(name="wot", bufs=32))
            cf_pool = ec(tc.tile_pool(name="cf", bufs=8))
            rot_pool = ec(tc.tile_pool(name="rot_scr", bufs=1))
            pt_pool = ec(tc.tile_pool(name="pt", bufs=4))
            bbsb_pool = ec(tc.tile_pool(name="bbsb", bufs=2))
            sums_pool = ec(tc.tile_pool(name="sums", bufs=1))
            cstg_pool = ec(tc.tile_pool(name="cstg", bufs=4))

            # first-half hsT loads go first so the first matmul starts ASAP
            hst0 = []
            for dt in range(32):
                ht = hst_pool.tile([P, HALFW], bf16, tag="hst", name="hst")
                nc.sync.dma_start(out=ht[:], in_=t["hsT"][dt * P:(dt + 1) * P,
                                                          0:HALFW])
                hst0.append(ht)

            ones_col = const_pool.tile([P, 1], bf16)
            nc.vector.memset(ones_col[:], 1.0)
            ones_row = const_pool.tile([1, P], f32)
            nc.vector.memset(ones_row[:], 1.0)
            cos_sb = const_pool.tile([NROT2, S], f32)
            nc.sync.dma_start(out=cos_sb[:], in_=t["cosT"][:])
            sin_sb = const_pool.tile([NROT2, S], f32)
            nc.sync.dma_start(out=sin_sb[:], in_=t["sinT"][:])
            mask_sb = const_pool.tile([P, 4, 512], f32)
            nc.sync.dma_start(out=mask_sb[:], in_=t["masks"][:])

            wot = []  # resident Wo^T column shard, loaded after P0 (below)

            qres = {}   # (et, scg) -> [128, 512] bf16  (q^T, rotary applied)
            kres = {}   # (et, scg) -> [128, 512] bf16  (k^T, rotary applied)
            vres = [None] * 16                      # [st16] -> [128s, 512e]

            def rot_evict(ps, stg, cols):
                # partitions [0:32) even pairs, [32:64) odd pairs, rest plain
                ca = cos_sb[:, cols:cols + 512]
                sa = sin_sb[:, cols:cols + 512]
                s1 = rot_pool.tile([NROT2, 512], f32, tag="rs1", name="rs1")
                s2 = rot_pool.tile([NROT2, 512], f32, tag="rs2", name="rs2")
                nc.vector.tensor_tensor(s1[:], ps[0:NROT2, :], ca, MUL)
                nc.vector.tensor_tensor(s2[:], ps[NROT2:ROT, :], sa, MUL)
                nc.vector.tensor_tensor(stg[0:NROT2, :], s1[:], s2[:], SUB)
                s3 = rot_pool.tile([NROT2, 512], f32, tag="rs1", name="rs1")
                s4 = rot_pool.tile([NROT2, 512], f32, tag="rs2", name="rs2")
                nc.vector.tensor_tensor(s3[:], ps[NROT2:ROT, :], ca, MUL)
                nc.vector.tensor_tensor(s4[:], ps[0:NROT2, :], sa, MUL)
                nc.vector.tensor_tensor(stg[NROT2:ROT, :], s3[:], s4[:], ADD)
                nc.scalar.activation(stg[ROT:P, :], ps[ROT:P, :], COPY)

            def evict(dst_ap, src_ps, on_act):
                if on_act:
                    nc.scalar.activation(dst_ap, src_ps, COPY)
                else:
                    nc.vector.tensor_copy(dst_ap, src_ps)

            # ================= Phase P: QKV projections for one half ==========
            def phase_P(half, hst, psum_p, parts=("q", "k", "v"),
                        v_dt_hook=None):
                c0 = half * HALFW
                # ---- Q then K: out layout [e, s], 2 waves of 4 banks ----
                for wname, is_q in (("wqT", True), ("wkT", False)):
                    if ("q" if is_q else "k") not in parts:
                        continue
                    acc = {}
                    for wave in range(2):
                        wacc = [psum_p.tile([P, 512], f32, tag="pp",
                                            name="pp") for _ in range(4)]
                        for dt in range(32):
                            wp = wpan_pool.tile([P, HD], bf16, tag="wp",
                                                name="wp")
                            nc.scalar.dma_start(
                                out=wp[:],
                                in_=t[wname][dt * P:(dt + 1) * P,
                                             wave * HD:(wave + 1) * HD])
                            for el in range(2):
                                for sc in range(2):
                                    nc.tensor.matmul(
                                        wacc[el * 2 + sc][:],
                                        wp[:, el * P:(el + 1) * P],
                                        hst[dt][:, sc * 512:(sc + 1) * 512],
                                        start=(dt == 0), stop=(dt == 31))
                        for el in range(2):
                            for sc in range(2):
                                acc[(wave * 2 + el) * 2 + sc] = wacc[el * 2 + sc]
                    for et in range(4):
                        for sc in range(2):
                            cols = c0 + sc * 512
                            scg = half * 2 + sc
                            ps = acc[et * 2 + sc]
                            dst = (qres_pool if is_q else kres_pool).tile(
                                [P, 512], bf16,
                                tag=("qres" if is_q else "kres"),
                                name=("qres" if is_q else "kres"))
                            if et in (0, 2):
                                rot_evict(ps, dst, cols)
                            else:
                                evict(dst[:], ps[:], on_act=(sc == 1))
                            (qres if is_q else kres)[(et, scg)] = dst

                # ---- V: natural layout [s, e] ----
                if "v" not in parts:
                    return
                acc = [psum_p.tile([P, 512], f32, tag="pp", name="pp")
                       for _ in range(8)]
                for dt in range(32):
                    wp = wpan_pool.tile([P, EL], bf16, tag="wp", name="wp")
                    nc.scalar.dma_start(
                        out=wp[:], in_=t["wvT"][dt * P:(dt + 1) * P, :])
                    for st in range(8):
                        nc.tensor.matmul(
                            acc[st][:],
                            hst[dt][:, st * P:(st + 1) * P],
                            wp[:],
                            start=(dt == 0), stop=(dt == 31))
                    if v_dt_hook is not None:
                        v_dt_hook(dt)
                for st in range(8):
                    vtile = vres_pool.tile([P, 512], bf16, tag="vres",
                                           name="vres")
                    evict(vtile[:], acc[st][:], on_act=(st % 2 == 1))
                    vres[half * 8 + st] = vtile

            # ============ Phase A: attention for one query group =============
            def phase_A(qg, psum_s, psum_c, psum_u, psum_b):
                nkb = 4 * qg + 4
                pending = []   # deferred per-head finalize (emitted after the
                               # next head's score prefetch to keep PE busy)

                def finalize(h, ctxp, sump):
                    e0 = h * HD
                    # normalization: broadcast sum, full-rate recip, scale
                    sum_sb = sums_pool.tile([1, 512], f32, tag="sums",
                                            name="sums")
                    nc.scalar.activation(sum_sb[:], sump[:], COPY)
                    bb = psum_b.tile([P, 512], f32, tag="pb", name="pb")
                    nc.tensor.matmul(bb[:], ones_row[:], sum_sb[:],
                                     start=True, stop=True)
                    rcp_bc = bbsb_pool.tile([P, 512], f32, tag="bbsb",
                                            name="bbsb")
                    nc.vector.reciprocal(rcp_bc[:], bb[:])
                    for half in range(2):
                        cst = cstg_pool.tile([P, 512], bf16, tag="cstg",
                                             name="cstg")
                        nc.vector.tensor_tensor(cst[:], ctxp[half][:],
                                                rcp_bc[:], MUL)
                        nc.gpsimd.dma_start(
                            out=t["ctxg"][qg][e0 + half * P:
                                              e0 + (half + 1) * P, :],
                            in_=cst[:])

                for h in range(HL):
                    e0 = h * HD
                    qt = [qres[(h * 2 + half, qg)] for half in range(2)]
                    ctxp = [psum_c.tile([P, 512], f32, tag="pc", name="pc")
                            for _ in range(2)]
                    sump = psum_u.tile([1, 512], f32, tag="pu", name="pu")
                    ps_list = [None] * nkb

                    def emit_scores(kb, h=h, qt=qt, ps_list=ps_list):
                        ps = psum_s.tile([P, 512], f32, tag="ps", name="ps")
                        for half in range(2):
                            nc.tensor.matmul(
                                ps[:],
                                kres[(h * 2 + half, kb // 4)][
                                    :, (kb % 4) * P:(kb % 4 + 1) * P],
                                qt[half][:],
                                start=(half == 0), stop=(half == 1))
                        ps_list[kb] = ps

                    for kb0 in range(min(3, nkb)):
                        emit_scores(kb0)
                    while pending:
                        pending.pop(0)()
                    for kb in range(nkb):
                        ps = ps_list[kb]
                        if kb >= 4 * qg:
                            m = kb - 4 * qg
                            nc.vector.tensor_tensor(
                                ps[:], ps[:], mask_sb[:, m, :], ADD)
                        p = pt_pool.tile([P, 512], bf16, tag="pt", name="pt")
                        nc.scalar.activation(p[:], ps[:], EXP)
                        if kb + 3 < nkb:
                            emit_scores(kb + 3)
                        nc.tensor.matmul(
                            sump[:], ones_col[:], p[:],
                            start=(kb == 0), stop=(kb == nkb - 1))
                        for half in range(2):
                            nc.tensor.matmul(
                                ctxp[half][:],
                                vres[kb][:, e0 + half * P:e0 + (half + 1) * P],
                                p[:],
                                start=(kb == 0), stop=(kb == nkb - 1))
                    pending.append(
                        lambda h=h, ctxp=ctxp, sump=sump: finalize(h, ctxp, sump))

                while pending:
                    pending.pop(0)()
                # ctx^T for this query group is complete -> AllGather it
                nc.gpsimd.collective_compute(
                    "AllGather",
                    mybir.AluOpType.bypass,
                    replica_groups=[list(range(NCORES))],
                    ins=[t["ctxg"][qg][:]],
                    outs=[t["ctxf"][qg][:]],
                )

            # ======== Phase O: out-projection rows [512qg, 512qg+512) ========
            cf_pre = {}   # qg -> prefetched leading cf tiles

            def prefetch_O(qg, n=8):
                tiles = []
                for dt in range(n):
                    ctile = cf_pool.tile([P, 512], bf16, tag="cf", name="cf")
                    nc.sync.dma_start(
                        out=ctile[:], in_=t["ctxf"][qg][dt * P:(dt + 1) * P, :])
                    tiles.append(ctile)
                cf_pre[qg] = tiles

            def phase_O(qg, psum_o):
                cf = list(cf_pre.pop(qg, []))
                for dt in range(len(cf), 32):
                    ctile = cf_pool.tile([P, 512], bf16, tag="cf", name="cf")
                    nc.sync.dma_start(
                        out=ctile[:], in_=t["ctxf"][qg][dt * P:(dt + 1) * P, :])
                    cf.append(ctile)
                po = [psum_o.tile([P, 512], f32, tag="po", name="po")
                      for _ in range(4)]
                for dt in range(32):
                    for st in range(4):
                        nc.tensor.matmul(
                            po[st][:],
                            cf[dt][:, st * P:(st + 1) * P],
                            wot[dt][:],
                            start=(dt == 0), stop=(dt == 31))
                for st in range(4):
                    stg = stage_pool.tile([P, 512], bf16, tag="stg",
                                          name="stg")
                    evict(stg[:], po[st][:], on_act=(st % 2 == 1))
                    nc.scalar.dma_start(
                        out=t["out"][qg * 512 + st * P:qg * 512 + (st + 1) * P, :],
                        in_=stg[:])

            # ============================ schedule ============================
            # hst half-1 tiles prefetch as soon as P0's V phase releases the
            # matching half-0 buffer (rings are free during late P0)
            hst1 = []

            def v_hook(dt):
                ht = hst_pool.tile([P, HALFW], bf16, tag="hst", name="hst")
                nc.sync.dma_start(
                    out=ht[:], in_=t["hsT"][dt * P:(dt + 1) * P, HALFW:S])
                hst1.append(ht)

            with tc.tile_pool(name="psum_p0", bufs=8, space="PSUM") as psum_p:
                phase_P(0, hst0, psum_p, v_dt_hook=v_hook)

            # Wo^T column shard: needed first at O0, loads during A0/A1
            for dt in range(32):
                wtile = wot_pool.tile([P, EL], bf16, tag="wot", name="wot")
                nc.sync.dma_start(out=wtile[:],
                                  in_=t["woT2"][dt * P:(dt + 1) * P, :])
                wot.append(wtile)

            with contextlib.ExitStack() as stk_a:
                eca = stk_a.enter_context
                psum_s = eca(tc.tile_pool(name="ps_s", bufs=3, space="PSUM"))
                psum_c = eca(tc.tile_pool(name="ps_c", bufs=2, space="PSUM"))
                psum_u = eca(tc.tile_pool(name="ps_u", bufs=1, space="PSUM"))
                psum_b = eca(tc.tile_pool(name="ps_b", bufs=1, space="PSUM"))
                phase_A(0, psum_s, psum_c, psum_u, psum_b)
                prefetch_O(0)
                phase_A(1, psum_s, psum_c, psum_u, psum_b)

            # O phases next: their cf loads read Shared scratch, which flows
            # even while an AllGather is in flight; P1's Local-DRAM weight
            # traffic would starve, so it waits until the collectives drain
            with tc.tile_pool(name="psum_o01", bufs=8, space="PSUM") as psum_o:
                phase_O(0, psum_o)
                phase_O(1, psum_o)

            with tc.tile_pool(name="psum_p1", bufs=8, space="PSUM") as psum_p:
                phase_P(1, hst1, psum_p)

            with contextlib.ExitStack() as stk_a:
                eca = stk_a.enter_context
                psum_s = eca(tc.tile_pool(name="ps_s2", bufs=3, space="PSUM"))
                psum_c = eca(tc.tile_pool(name="ps_c2", bufs=2, space="PSUM"))
                psum_u = eca(tc.tile_pool(name="ps_u2", bufs=1, space="PSUM"))
                psum_b = eca(tc.tile_pool(name="ps_b2", bufs=1, space="PSUM"))
                phase_A(2, psum_s, psum_c, psum_u, psum_b)
                prefetch_O(2)
                phase_A(3, psum_s, psum_c, psum_u, psum_b)

            with tc.tile_pool(name="psum_o23", bufs=8, space="PSUM") as psum_o:
                phase_O(2, psum_o)
                phase_O(3, psum_o)


def _build():
    if "nc" in _CACHE:
        return _CACHE["nc"]
    nc = bacc.Bacc(None, num_devices=NCORES)
    t = {}
    t["hsT"] = nc.declare_dram_parameter("hsT", [D, S], bf16, isOutput=False)
    t["wqT"] = nc.declare_dram_parameter("wqT", [D, EL], bf16, isOutput=False)
    t["wkT"] = nc.declare_dram_parameter("wkT", [D, EL], bf16, isOutput=False)
    t["wvT"] = nc.declare_dram_parameter("wvT", [D, EL], bf16, isOutput=False)
    t["woT2"] = nc.declare_dram_parameter("woT2", [D, EL], bf16, isOutput=False)
    t["cosT"] = nc.declare_dram_parameter("cosT", [NROT2, S], f32, isOutput=False)
    t["sinT"] = nc.declare_dram_parameter("sinT", [NROT2, S], f32, isOutput=False)
    t["masks"] = nc.declare_dram_parameter("masks", [P, 4, 512], f32, isOutput=False)
    t["out"] = nc.declare_dram_parameter("out", [S, EL], bf16, isOutput=True)
    t["ctxg"] = [nc.dram_tensor(f"ctxg{qg}", [EL, 512], bf16)
                 for qg in range(4)]
    t["ctxf"] = [nc.dram_tensor(f"ctxf{qg}", [D, 512], bf16,
                                addr_space="Shared")
                 for qg in range(4)]

    _emit(nc, t)
    nc.compile()
    _CACHE["nc"] = nc
    return nc


def _prep_inputs(hidden_states, Wq, Wk, Wv, Wo, attention_mask, position_ids):
    hs = np.asarray(hidden_states, np.float32).reshape(S, D)
    hsT = np.ascontiguousarray(hs.T).astype(ml_dtypes.bfloat16)

    pos = np.asarray(position_ids).reshape(S).astype(np.float32)
    inv = 10000.0 ** (-np.arange(0, ROT, 2, dtype=np.float32) / ROT)  # [32]
    ang = pos[:, None] * inv[None, :]                                  # [S, 32]
    cosT = np.ascontiguousarray(np.cos(ang).T).astype(np.float32)
    sinT = np.ascontiguousarray(np.sin(ang).T).astype(np.float32)

    am = np.asarray(attention_mask, np.float32).reshape(S, S)
    masks = np.empty((P, 4, 512), np.float32)
    for m in range(4):
        # transposed-score layout: mask[p, n] for k = m*128+p, q = n
        masks[:, m, :] = am[0:512, m * P:(m + 1) * P].T

    # within-head row permutation: even rot dims, odd rot dims, the rest
    perm1 = np.concatenate([np.arange(0, ROT, 2), np.arange(1, ROT, 2),
                            np.arange(ROT, HD)])
    perm = np.concatenate([perm1 + HD * j for j in range(HL)])

    Wq = np.asarray(Wq, np.float32)
    Wk = np.asarray(Wk, np.float32)
    Wv = np.asarray(Wv, np.float32)
    Wo = np.asarray(Wo, np.float32)
    scale = 1.0 / np.sqrt(np.float32(HD))

    in_maps = []
    for c in range(NCORES):
        rows = slice(c * EL, (c + 1) * EL)
        wq_c = Wq[rows][perm] * scale
        wk_c = Wk[rows][perm]
        wv_c = Wv[rows]
        in_maps.append({
            "hsT": hsT,
            "wqT": np.ascontiguousarray(wq_c.T).astype(ml_dtypes.bfloat16),
            "wkT": np.ascontiguousarray(wk_c.T).astype(ml_dtypes.bfloat16),
            "wvT": np.ascontiguousarray(wv_c.T).astype(ml_dtypes.bfloat16),
            "woT2": np.ascontiguousarray(Wo[rows, :].T).astype(ml_dtypes.bfloat16),
            "cosT": cosT,
            "sinT": sinT,
            "masks": masks,
        })
    return in_maps


def run(inputs, trace=False):
    """Run on HW. Returns (full_output, BassKernelResults)."""
    nc = _build()
    in_maps = _prep_inputs(**inputs)
    res = run_bass_kernel_spmd(nc, in_maps, list(range(NCORES)), trace=trace)
    # core c computed output columns [c*EL, (c+1)*EL)
    full = np.empty((S, D), np.float32)
    for c in range(NCORES):
        full[:, c * EL:(c + 1) * EL] = \
            np.asarray(res.results[c]["out"]).astype(np.float32)
    return full.reshape(B, S, D), res


def kernel(**inputs):
    full, _ = run(inputs, trace=False)
    return full


# revision 32
# speedup vs baseline: 1.0455x; 1.0455x over previous
"""GPT-J attention (B=1, S=2048, D=4096, H=16, HD=256, rot=64) on 8 TRN2 cores.

Strategy: tensor-parallel over heads (2 heads/core) for QKV+attention, then
column-parallel out-projection: after attention, each core AllGathers the full
ctx^T (its [512, 512] per query group -> shared [4096, 512]) and computes a
complete 512-column slice of the output (contracting the full d=4096), so no
ReduceScatter of 16.8MB partials is needed -- the host just concatenates the
8 column shards. Collective volume drops 8x and the reduction disappears.

Phases are interleaved so the AllGathers hide under PE work and the kernel
ends on matmuls, not comms:
  P0 (QKV for s<1024) -> A0,A1 (+AG0,AG1) -> O0,O1 -> P1 -> A2,A3 -> O2,O3

Host-side prep (cheap, numpy):
  - hsT = hidden_states.T so all device matmuls contract over the partition dim
  - per-core weight shards pre-transposed; Wq/Wk rows permuted within each head
    (even rot dims, odd rot dims, rest) so rotary becomes plain block ops
  - woT2 = Wo[cols of this core, :].T  (column shard of the out-projection)
  - 1/sqrt(HD) folded into Wq; sin/cos tables and causal mask tiles precomputed

Matmul operands are bf16 (fp32r measured 2 cyc/row on HW; bf16 is 1), all
accumulation in fp32 PSUM; softmax exp input, masks, rotary and the softmax
normalization run in fp32. q/K^T/V stay SBUF-resident between phases; ctx^T
roundtrips through DRAM via the AllGather. PSUM evictions alternate DVE/ACT.
"""

import contextlib
import numpy as np
import ml_dtypes

import concourse.bass as bass
import concourse.tile as tile
import concourse.mybir as mybir
from concourse import bacc
from concourse.bass_utils import run_bass_kernel_spmd

B, S, D = 1, 2048, 4096
H, HD, ROT = 16, 256, 64
NCORES = 8
HL = H // NCORES          # heads per core = 2
EL = D // NCORES          # local e width = 512
HALFW = S // 2            # 1024
P = 128
NROT2 = ROT // 2          # 32

f32 = mybir.dt.float32
bf16 = mybir.dt.bfloat16
EXP = mybir.ActivationFunctionType.Exp
COPY = mybir.ActivationFunctionType.Copy
ADD = mybir.AluOpType.add
MUL = mybir.AluOpType.mult
SUB = mybir.AluOpType.subtract

_CACHE = {}


def _emit(nc, t):
    """Emit the whole per-core program inside a TileContext."""
    with tile.TileContext(nc) as tc:
        with contextlib.ExitStack() as _stk:
            ec = _stk.enter_context
            const_pool = ec(tc.tile_pool(name="const", bufs=1))
            wpan_pool = ec(tc.tile_pool(name="wpan", bufs=4))
            stage_pool = ec(tc.tile_pool(name="stage", bufs=4))
            hst_pool = ec(tc.tile_pool(name="hst", bufs=32))
            qres_pool = ec(tc.tile_pool(name="qres", bufs=16))
            kres_pool = ec(tc.tile_pool(name="kres", bufs=16))
            vres_pool = ec(tc.tile_pool(name="vres", bufs=16))
            wot_pool = ec(tc.tile_pool(name="wot", bufs=32))
            cf_pool = ec(tc.tile_pool(name="cf", bufs=8))
            rot_pool = ec(tc.tile_pool(name="rot_scr", bufs=1))
            pt_pool = ec(tc.tile_pool(name="pt", bufs=4))
            bbsb_pool = ec(tc.tile_pool(name="bbsb", bufs=2))
            sums_pool = ec(tc.tile_pool(name="sums", bufs=1))
            cstg_pool = ec(tc.tile_pool(name="cstg", bufs=4))

            # first-half hsT loads go first so the first matmul starts ASAP
            hst0 = []
            for dt in range(32):
                ht = hst_pool.tile([P, HALFW], bf16, tag="hst", name="hst")
                nc.sync.dma_start(out=ht[:], in_=t["hsT"][dt * P:(dt + 1) * P,
                                                          0:HALFW])
                hst0.append(ht)

            ones_col = const_pool.tile([P, 1], bf16)
            nc.vector.memset(ones_col[:], 1.0)
            ones_row = const_pool.tile([1, P], f32)
            nc.vector.memset(ones_row[:], 1.0)
            cos_sb = const_pool.tile([NROT2, S], f32)
            nc.sync.dma_start(out=cos_sb[:], in_=t["cosT"][:])
            sin_sb = const_pool.tile([NROT2, S], f32)
            nc.sync.dma_start(out=sin_sb[:], in_=t["sinT"][:])
            mask_sb = const_pool.tile([P, 4, 512], f32)
            nc.sync.dma_start(out=mask_sb[:], in_=t["masks"][:])

            wot = []  # resident Wo^T column shard, loaded after P0 (below)

            qres = {}   # (et, scg) -> [128, 512] bf16  (q^T, rotary applied)
            kres = {}   # (et, scg) -> [128, 512] bf16  (k^T, rotary applied)
            vres = [None] * 16                      # [st16] -> [128s, 512e]

            def rot_evict(ps, stg, cols):
                # partitions [0:32) even pairs, [32:64) odd pairs, rest plain
                ca = cos_sb[:, cols:cols + 512]
                sa = sin_sb[:, cols:cols + 512]
                s1 = rot_pool.tile([NROT2, 512], f32, tag="rs1", name="rs1")
                s2 = rot_pool.tile([NROT2, 512], f32, tag="rs2", name="rs2")
                nc.vector.tensor_tensor(s1[:], ps[0:NROT2, :], ca, MUL)
                nc.vector.tensor_tensor(s2[:], ps[NROT2:ROT, :], sa, MUL)
                nc.vector.tensor_tensor(stg[0:NROT2, :], s1[:], s2[:], SUB)
                s3 = rot_pool.tile([NROT2, 512], f32, tag="rs1", name="rs1")
                s4 = rot_pool.tile([NROT2, 512], f32, tag="rs2", name="rs2")
                nc.vector.tensor_tensor(s3[:], ps[NROT2:ROT, :], ca, MUL)
                nc.vector.tensor_tensor(s4[:], ps[0:NROT2, :], sa, MUL)
                nc.vector.tensor_tensor(stg[NROT2:ROT, :], s3[:], s4[:], ADD)
                nc.scalar.activation(stg[ROT:P, :], ps[ROT:P, :], COPY)

            def evict(dst_ap, src_ps, on_act):
                if on_act:
                    nc.scalar.activation(dst_ap, src_ps, COPY)
                else:
                    nc.vector.tensor_copy(dst_ap, src_ps)

            # ================= Phase P: QKV projections for one half ==========
            def phase_P(half, hst, psum_p, parts=("q", "k", "v"),
                        v_dt_hook=None):
                c0 = half * HALFW
                # ---- Q then K: out layout [e, s], 2 waves of 4 banks ----
                for wname, is_q in (("wqT", True), ("wkT", False)):
                    if ("q" if is_q else "k") not in parts:
                        continue
                    acc = {}
                    for wave in range(2):
                        wacc = [psum_p.tile([P, 512], f32, tag="pp",
                                            name="pp") for _ in range(4)]
                        for dt in range(32):
                            wp = wpan_pool.tile([P, HD], bf16, tag="wp",
                                                name="wp")
                            nc.scalar.dma_start(
                                out=wp[:],
                                in_=t[wname][dt * P:(dt + 1) * P,
                                             wave * HD:(wave + 1) * HD])
                            for el in range(2):
                                for sc in range(2):
                                    nc.tensor.matmul(
                                        wacc[el * 2 + sc][:],
                                        wp[:, el * P:(el + 1) * P],
                                        hst[dt][:, sc * 512:(sc + 1) * 512],
                                        start=(dt == 0), stop=(dt == 31))
                        for el in range(2):
                            for sc in range(2):
                                acc[(wave * 2 + el) * 2 + sc] = wacc[el * 2 + sc]
                    for et in range(4):
                        for sc in range(2):
                            cols = c0 + sc * 512
                            scg = half * 2 + sc
                            ps = acc[et * 2 + sc]
                            dst = (qres_pool if is_q else kres_pool).tile(
                                [P, 512], bf16,
                                tag=("qres" if is_q else "kres"),
                                name=("qres" if is_q else "kres"))
                            if et in (0, 2):
                                rot_evict(ps, dst, cols)
                            else:
                                evict(dst[:], ps[:], on_act=(sc == 1))
                            (qres if is_q else kres)[(et, scg)] = dst

                # ---- V: natural layout [s, e] ----
                if "v" not in parts:
                    return
                acc = [psum_p.tile([P, 512], f32, tag="pp", name="pp")
                       for _ in range(8)]
                for dt in range(32):
                    wp = wpan_pool.tile([P, EL], bf16, tag="wp", name="wp")
                    nc.scalar.dma_start(
                        out=wp[:], in_=t["wvT"][dt * P:(dt + 1) * P, :])
                    for st in range(8):
                        nc.tensor.matmul(
                            acc[st][:],
                            hst[dt][:, st * P:(st + 1) * P],
                            wp[:],
                            start=(dt == 0), stop=(dt == 31))
                    if v_dt_hook is not None:
                        v_dt_hook(dt)
                for st in range(8):
                    vtile = vres_pool.tile([P, 512], bf16, tag="vres",
                                           name="vres")
                    evict(vtile[:], acc[st][:], on_act=(st % 2 == 1))
                    vres[half * 8 + st] = vtile

            # ============ Phase A: attention for one query group =============
            def phase_A(qg, psum_s, psum_c, psum_u, psum_b):
                nkb = 4 * qg + 4
                pending = []   # deferred per-head finalize (emitted after the
                               # next head's score prefetch to keep PE busy)

                def finalize(h, ctxp, sump):
                    e0 = h * HD
                    # normalization: broadcast sum, full-rate recip, scale
                    sum_sb = sums_pool.tile([1, 512], f32, tag="sums",
                                            name="sums")
                    nc.scalar.activation(sum_sb[:], sump[:], COPY)
                    bb = psum_b.tile([P, 512], f32, tag="pb", name="pb")
                    nc.tensor.matmul(bb[:], ones_row[:], sum_sb[:],
                                     start=True, stop=True)
                    rcp_bc = bbsb_pool.tile([P, 512], f32, tag="bbsb",
                                            name="bbsb")
                    nc.vector.reciprocal(rcp_bc[:], bb[:])
                    for half in range(2):
                        cst = cstg_pool.tile([P, 512], bf16, tag="cstg",
                                             name="cstg")
                        nc.vector.tensor_tensor(cst[:], ctxp[half][:],
                                                rcp_bc[:], MUL)
                        nc.gpsimd.dma_start(
                            out=t["ctxg"][qg][e0 + half * P:
                                              e0 + (half + 1) * P, :],
                            in_=cst[:])

                for h in range(HL):
                    e0 = h * HD
                    qt = [qres[(h * 2 + half, qg)] for half in range(2)]
                    ctxp = [psum_c.tile([P, 512], f32, tag="pc", name="pc")
                            for _ in range(2)]
                    sump = psum_u.tile([1, 512], f32, tag="pu", name="pu")
                    ps_list = [None] * nkb

                    def emit_scores(kb, h=h, qt=qt, ps_list=ps_list):
                        ps = psum_s.tile([P, 512], f32, tag="ps", name="ps")
                        for half in range(2):
                            nc.tensor.matmul(
                                ps[:],
                                kres[(h * 2 + half, kb // 4)][
                                    :, (kb % 4) * P:(kb % 4 + 1) * P],
                                qt[half][:],
                                start=(half == 0), stop=(half == 1))
                        ps_list[kb] = ps

                    for kb0 in range(min(3, nkb)):
                        emit_scores(kb0)
                    while pending:
                        pending.pop(0)()
                    for kb in range(nkb):
                        ps = ps_list[kb]
                        if kb >= 4 * qg:
                            m = kb - 4 * qg
                            nc.vector.tensor_tensor(
                                ps[:], ps[:], mask_sb[:, m, :], ADD)
                        p = pt_pool.tile([P, 512], bf16, tag="pt", name="pt")
                        nc.scalar.activation(p[:], ps[:], EXP)
                        if kb + 3 < nkb:
                            emit_scores(kb + 3)
                        nc.tensor.matmul(
                            sump[:], ones_col[:], p[:],
                            start=(kb == 0), stop=(kb == nkb - 1))
                        for half in range(2):
                            nc.tensor.matmul(
                                ctxp[half][:],
                                vres[kb][:, e0 + half * P:e0 + (half + 1) * P],
                                p[:],
                                start=(kb == 0), stop=(kb == nkb - 1))
                    pending.append(
                        lambda h=h, ctxp=ctxp, sump=sump: finalize(h, ctxp, sump))

                while pending:
                    pending.pop(0)()
                # ctx^T for this query group is complete -> AllGather it
                nc.gpsimd.collective_compute(
                    "AllGather",
                    mybir.AluOpType.bypass,
                    replica_groups=[list(range(NCORES))],
                    ins=[t["ctxg"][qg][:]],
                    outs=[t["ctxf"][qg][:]],
                )

            # ======== Phase O: out-projection rows [512qg, 512qg+512) ========
            cf_pre = {}   # qg -> prefetched leading cf tiles

            def prefetch_O(qg, n=8):
                tiles = []
                for dt in range(n):
                    ctile = cf_pool.tile([P, 512], bf16, tag="cf", name="cf")
                    nc.sync.dma_start(
                        out=ctile[:], in_=t["ctxf"][qg][dt * P:(dt + 1) * P, :])
                    tiles.append(ctile)
                cf_pre[qg] = tiles

            def phase_O(qg, psum_o):
                cf = list(cf_pre.pop(qg, []))
                for dt in range(len(cf), 32):
                    ctile = cf_pool.tile([P, 512], bf16, tag="cf", name="cf")
                    nc.sync.dma_start(
                        out=ctile[:], in_=t["ctxf"][qg][dt * P:(dt + 1) * P, :])
                    cf.append(ctile)
                po = [psum_o.tile([P, 512], f32, tag="po", name="po")
                      for _ in range(4)]
                for dt in range(32):
                    for st in range(4):
                        nc.tensor.matmul(
                            po[st][:],
                            cf[dt][:, st * P:(st + 1) * P],
                            wot[dt][:],
                            start=(dt == 0), stop=(dt == 31))
                for st in range(4):
                    stg = stage_pool.tile([P, 512], bf16, tag="stg",
                                          name="stg")
                    evict(stg[:], po[st][:], on_act=(st % 2 == 1))
                    nc.scalar.dma_start(
                        out=t["out"][qg * 512 + st * P:qg * 512 + (st + 1) * P, :],
                        in_=stg[:])

            # ============================ schedule ============================
            # hst half-1 tiles prefetch as soon as P0's V phase releases the
            # matching half-0 buffer (rings are free during late P0)
            hst1 = []

            def v_hook(dt):
                ht = hst_pool.tile([P, HALFW], bf16, tag="hst", name="hst")
                nc.sync.dma_start(
                    out=ht[:], in_=t["hsT"][dt * P:(dt + 1) * P, HALFW:S])
                hst1.append(ht)

            with tc.tile_pool(name="psum_p0", bufs=8, space="PSUM") as psum_p:
                phase_P(0, hst0, psum_p, v_dt_hook=v_hook)

            # Wo^T column shard: needed first at O0, loads during A0/A1
            for dt in range(32):
                wtile = wot_pool.tile([P, EL], bf16, tag="wot", name="wot")
                nc.sync.dma_start(out=wtile[:],
                                  in_=t["woT2"][dt * P:(dt + 1) * P, :])
                wot.append(wtile)

            with contextlib.ExitStack() as stk_a:
                eca = stk_a.enter_context
                psum_s = eca(tc.tile_pool(name="ps_s", bufs=3, space="PSUM"))
                psum_c = eca(tc.tile_pool(name="ps_c", bufs=2, space="PSUM"))
                psum_u = eca(tc.tile_pool(name="ps_u", bufs=1, space="PSUM"))
                psum_b = eca(tc.tile_pool(name="ps_b", bufs=1, space="PSUM"))
                phase_A(0, psum_s, psum_c, psum_u, psum_b)
                phase_A(1, psum_s, psum_c, psum_u, psum_b)

            # O phases next: their cf loads read Shared scratch, which flows
            # even while an AllGather is in flight; P1's Local-DRAM weight
            # traffic would starve, so it waits until the collectives drain
            with tc.tile_pool(name="psum_o01", bufs=8, space="PSUM") as psum_o:
                phase_O(0, psum_o)
                phase_O(1, psum_o)

            with tc.tile_pool(name="psum_p1", bufs=8, space="PSUM") as psum_p:
                phase_P(1, hst1, psum_p)

            with contextlib.ExitStack() as stk_a:
                eca = stk_a.enter_context
                psum_s = eca(tc.tile_pool(name="ps_s2", bufs=3, space="PSUM"))
                psum_c = eca(tc.tile_pool(name="ps_c2", bufs=2, space="PSUM"))
                psum_u = eca(tc.tile_pool(name="ps_u2", bufs=1, space="PSUM"))
                psum_b = eca(tc.tile_pool(name="ps_b2", bufs=1, space="PSUM"))
                phase_A(2, psum_s, psum_c, psum_u, psum_b)
                phase_A(3, psum_s, psum_c, psum_u, psum_b)

            with tc.tile_pool(name="psum_o23", bufs=8, space="PSUM") as psum_o:
                phase_O(2, psum_o)
                phase_O(3, psum_o)


def _build():
    if "nc" in _CACHE:
        return _CACHE["nc"]
    nc = bacc.Bacc(None, num_devices=NCORES)
    t = {}
    t["hsT"] = nc.declare_dram_parameter("hsT", [D, S], bf16, isOutput=False)
    t["wqT"] = nc.declare_dram_parameter("wqT", [D, EL], bf16, isOutput=False)
    t["wkT"] = nc.declare_dram_parameter("wkT", [D, EL], bf16, isOutput=False)
    t["wvT"] = nc.declare_dram_parameter("wvT", [D, EL], bf16, isOutput=False)
    t["woT2"] = nc.declare_dram_parameter("woT2", [D, EL], bf16, isOutput=False)
    t["cosT"] = nc.declare_dram_parameter("cosT", [NROT2, S], f32, isOutput=False)
    t["sinT"] = nc.declare_dram_parameter("sinT", [NROT2, S], f32, isOutput=False)
    t["masks"] = nc.declare_dram_parameter("masks", [P, 4, 512], f32, isOutput=False)
    t["out"] = nc.declare_dram_parameter("out", [S, EL], bf16, isOutput=True)
    t["ctxg"] = [nc.dram_tensor(f"ctxg{qg}", [EL, 512], bf16)
                 for qg in range(4)]
    t["ctxf"] = [nc.dram_tensor(f"ctxf{qg}", [D, 512], bf16,
                                addr_space="Shared")
                 for qg in range(4)]

    _emit(nc, t)
    nc.compile()
    _CACHE["nc"] = nc
    return nc


def _prep_inputs(hidden_states, Wq, Wk, Wv, Wo, attention_mask, position_ids):
    hs = np.asarray(hidden_states, np.float32).reshape(S, D)
    hsT = np.ascontiguousarray(hs.T).astype(ml_dtypes.bfloat16)

    pos = np.asarray(position_ids).reshape(S).astype(np.float32)
    inv = 10000.0 ** (-np.arange(0, ROT, 2, dtype=np.float32) / ROT)  # [32]
    ang = pos[:, None] * inv[None, :]                                  # [S, 32]
    cosT = np.ascontiguousarray(np.cos(ang).T).astype(np.float32)
    sinT = np.ascontiguousarray(np.sin(ang).T).astype(np.float32)

    am = np.asarray(attention_mask, np.float32).reshape(S, S)
    masks = np.empty((P, 4, 512), np.float32)
    for m in range(4):
        # transposed-score layout: mask[p, n] for k = m*128+p, q = n
        masks[:, m, :] = am[0:512, m * P:(m + 1) * P].T

    # within-head row permutation: even rot dims, odd rot dims, the rest
    perm1 = np.concatenate([np.arange(0, ROT, 2), np.arange(1, ROT, 2),
                            np.arange(ROT, HD)])
    perm = np.concatenate([perm1 + HD * j for j in range(HL)])

    Wq = np.asarray(Wq, np.float32)
    Wk = np.asarray(Wk, np.float32)
    Wv = np.asarray(Wv, np.float32)
    Wo = np.asarray(Wo, np.float32)
    scale = 1.0 / np.sqrt(np.float32(HD))

    in_maps = []
    for c in range(NCORES):
        rows = slice(c * EL, (c + 1) * EL)
        wq_c = Wq[rows][perm] * scale
        wk_c = Wk[rows][perm]
        wv_c = Wv[rows]
        in_maps.append({
            "hsT": hsT,
            "wqT": np.ascontiguousarray(wq_c.T).astype(ml_dtypes.bfloat16),
            "wkT": np.ascontiguousarray(wk_c.T).astype(ml_dtypes.bfloat16),
            "wvT": np.ascontiguousarray(wv_c.T).astype(ml_dtypes.bfloat16),
            "woT2": np.ascontiguousarray(Wo[rows, :].T).astype(ml_dtypes.bfloat16),
            "cosT": cosT,
            "sinT": sinT,
            "masks": masks,
        })
    return in_maps


def run(inputs, trace=False):
    """Run on HW. Returns (full_output, BassKernelResults)."""
    nc = _build()
    in_maps = _prep_inputs(**inputs)
    res = run_bass_kernel_spmd(nc, in_maps, list(range(NCORES)), trace=trace)
    # core c computed output columns [c*EL, (c+1)*EL)
    full = np.empty((S, D), np.float32)
    for c in range(NCORES):
        full[:, c * EL:(c + 1) * EL] = \
            np.asarray(res.results[c]["out"]).astype(np.float32)
    return full.reshape(B, S, D), res


def kernel(**inputs):
    full, _ = run(inputs, trace=False)
    return full


# revision 35
# speedup vs baseline: 1.1116x; 1.0633x over previous
"""GPT-J attention (B=1, S=2048, D=4096, H=16, HD=256, rot=64) on 8 TRN2 cores.

Strategy: tensor-parallel over heads (2 heads/core) for QKV+attention, then
column-parallel out-projection: after attention, each core AllGathers the full
ctx^T (its [512, 512] per query group -> shared [4096, 512]) and computes a
complete 512-column slice of the output (contracting the full d=4096), so no
ReduceScatter of 16.8MB partials is needed -- the host just concatenates the
8 column shards. Collective volume drops 8x and the reduction disappears.

Phases are interleaved so the AllGathers hide under PE work and the kernel
ends on matmuls, not comms:
  P0 (QKV for s<1024) -> A0,A1 (+AG0,AG1) -> O0,O1 -> P1 -> A2,A3 -> O2,O3

Host-side prep (cheap, numpy):
  - hsT = hidden_states.T so all device matmuls contract over the partition dim
  - per-core weight shards pre-transposed; Wq/Wk rows permuted within each head
    (even rot dims, odd rot dims, rest) so rotary becomes plain block ops
  - woT2 = Wo[cols of this core, :].T  (column shard of the out-projection)
  - 1/sqrt(HD) folded into Wq; sin/cos tables and causal mask tiles precomputed

Matmul operands are bf16 (fp32r measured 2 cyc/row on HW; bf16 is 1), all
accumulation in fp32 PSUM; softmax exp input, masks, rotary and the softmax
normalization run in fp32. q/K^T/V stay SBUF-resident between phases; ctx^T
roundtrips through DRAM via the AllGather. PSUM evictions alternate DVE/ACT.
"""

import contextlib
import numpy as np
import ml_dtypes

import concourse.bass as bass
import concourse.tile as tile
import concourse.mybir as mybir
from concourse import bacc
from concourse.bass_utils import run_bass_kernel_spmd

B, S, D = 1, 2048, 4096
H, HD, ROT = 16, 256, 64
NCORES = 8
HL = H // NCORES          # heads per core = 2
EL = D // NCORES          # local e width = 512
HALFW = S // 2            # 1024
P = 128
NROT2 = ROT // 2          # 32

f32 = mybir.dt.float32
bf16 = mybir.dt.bfloat16
EXP = mybir.ActivationFunctionType.Exp
COPY = mybir.ActivationFunctionType.Copy
ADD = mybir.AluOpType.add
MUL = mybir.AluOpType.mult
SUB = mybir.AluOpType.subtract

_CACHE = {}


def _emit(nc, t):
    """Emit the whole per-core program inside a TileContext."""
    with tile.TileContext(nc) as tc:
        with contextlib.ExitStack() as _stk:
            ec = _stk.enter_context
            const_pool = ec(tc.tile_pool(name="const", bufs=1))
            wpan_pool = ec(tc.tile_pool(name="wpan", bufs=4))
            stage_pool = ec(tc.tile_pool(name="stage", bufs=4))
            hst_pool = ec(tc.tile_pool(name="hst", bufs=32))
            qres_pool = ec(tc.tile_pool(name="qres", bufs=16))
            kres_pool = ec(tc.tile_pool(name="kres", bufs=16))
            vres_pool = ec(tc.tile_pool(name="vres", bufs=16))
            wot_pool = ec(tc.tile_pool(name="wot", bufs=32))
            cf_pool = ec(tc.tile_pool(name="cf", bufs=8))
            rot_pool = ec(tc.tile_pool(name="rot_scr", bufs=1))
            pt_pool = ec(tc.tile_pool(name="pt", bufs=4))
            bbsb_pool = ec(tc.tile_pool(name="bbsb", bufs=2))
            sums_pool = ec(tc.tile_pool(name="sums", bufs=1))
            cstg_pool = ec(tc.tile_pool(name="cstg", bufs=4))

            # first-half hsT loads go first so the first matmul starts ASAP
            hst0 = []
            for dt in range(32):
                ht = hst_pool.tile([P, HALFW], bf16, tag="hst", name="hst")
                nc.sync.dma_start(out=ht[:], in_=t["hsT"][dt * P:(dt + 1) * P,
                                                          0:HALFW])
                hst0.append(ht)

            ones_col = const_pool.tile([P, 1], bf16)
            nc.vector.memset(ones_col[:], 1.0)
            ones_row = const_pool.tile([1, P], f32)
            nc.vector.memset(ones_row[:], 1.0)
            cos_sb = const_pool.tile([NROT2, S], f32)
            nc.sync.dma_start(out=cos_sb[:], in_=t["cosT"][:])
            sin_sb = const_pool.tile([NROT2, S], f32)
            nc.sync.dma_start(out=sin_sb[:], in_=t["sinT"][:])
            mask_sb = const_pool.tile([P, 4, 512], f32)
            nc.sync.dma_start(out=mask_sb[:], in_=t["masks"][:])

            wot = []  # resident Wo^T column shard, loaded after P0 (below)

            qres = {}   # (et, scg) -> [128, 512] bf16  (q^T, rotary applied)
            kres = {}   # (et, scg) -> [128, 512] bf16  (k^T, rotary applied)
            vres = [None] * 16                      # [st16] -> [128s, 512e]

            def rot_evict(ps, stg, cols):
                # partitions [0:32) even pairs, [32:64) odd pairs, rest plain
                ca = cos_sb[:, cols:cols + 512]
                sa = sin_sb[:, cols:cols + 512]
                s1 = rot_pool.tile([NROT2, 512], f32, tag="rs1", name="rs1")
                s2 = rot_pool.tile([NROT2, 512], f32, tag="rs2", name="rs2")
                nc.vector.tensor_tensor(s1[:], ps[0:NROT2, :], ca, MUL)
                nc.vector.tensor_tensor(s2[:], ps[NROT2:ROT, :], sa, MUL)
                nc.vector.tensor_tensor(stg[0:NROT2, :], s1[:], s2[:], SUB)
                s3 = rot_pool.tile([NROT2, 512], f32, tag="rs1", name="rs1")
                s4 = rot_pool.tile([NROT2, 512], f32, tag="rs2", name="rs2")
                nc.vector.tensor_tensor(s3[:], ps[NROT2:ROT, :], ca, MUL)
                nc.vector.tensor_tensor(s4[:], ps[0:NROT2, :], sa, MUL)
                nc.vector.tensor_tensor(stg[NROT2:ROT, :], s3[:], s4[:], ADD)
                nc.scalar.activation(stg[ROT:P, :], ps[ROT:P, :], COPY)

            def evict(dst_ap, src_ps, on_act):
                if on_act:
                    nc.scalar.activation(dst_ap, src_ps, COPY)
                else:
                    nc.vector.tensor_copy(dst_ap, src_ps)

            # ================= Phase P: QKV projections for one half ==========
            def phase_P(half, hst, psum_p, parts=("q", "k", "v"),
                        v_dt_hook=None):
                c0 = half * HALFW
                # ---- Q then K: out layout [e, s], 2 waves of 4 banks ----
                for wname, is_q in (("wqT", True), ("wkT", False)):
                    if ("q" if is_q else "k") not in parts:
                        continue
                    acc = {}
                    for wave in range(2):
                        wacc = [psum_p.tile([P, 512], f32, tag="pp",
                                            name="pp") for _ in range(4)]
                        for dt in range(32):
                            wp = wpan_pool.tile([P, HD], bf16, tag="wp",
                                                name="wp")
                            nc.scalar.dma_start(
                                out=wp[:],
                                in_=t[wname][dt * P:(dt + 1) * P,
                                             wave * HD:(wave + 1) * HD])
                            for el in range(2):
                                for sc in range(2):
                                    nc.tensor.matmul(
                                        wacc[el * 2 + sc][:],
                                        wp[:, el * P:(el + 1) * P],
                                        hst[dt][:, sc * 512:(sc + 1) * 512],
                                        start=(dt == 0), stop=(dt == 31))
                        for el in range(2):
                            for sc in range(2):
                                acc[(wave * 2 + el) * 2 + sc] = wacc[el * 2 + sc]
                    for et in range(4):
                        for sc in range(2):
                            cols = c0 + sc * 512
                            scg = half * 2 + sc
                            ps = acc[et * 2 + sc]
                            dst = (qres_pool if is_q else kres_pool).tile(
                                [P, 512], bf16,
                                tag=("qres" if is_q else "kres"),
                                name=("qres" if is_q else "kres"))
                            if et in (0, 2):
                                rot_evict(ps, dst, cols)
                            else:
                                evict(dst[:], ps[:], on_act=(sc == 1))
                            (qres if is_q else kres)[(et, scg)] = dst

                # ---- V: natural layout [s, e] ----
                if "v" not in parts:
                    return
                acc = [psum_p.tile([P, 512], f32, tag="pp", name="pp")
                       for _ in range(8)]
                for dt in range(32):
                    wp = wpan_pool.tile([P, EL], bf16, tag="wp", name="wp")
                    nc.scalar.dma_start(
                        out=wp[:], in_=t["wvT"][dt * P:(dt + 1) * P, :])
                    for st in range(8):
                        nc.tensor.matmul(
                            acc[st][:],
                            hst[dt][:, st * P:(st + 1) * P],
                            wp[:],
                            start=(dt == 0), stop=(dt == 31))
                    if v_dt_hook is not None:
                        v_dt_hook(dt)
                for st in range(8):
                    vtile = vres_pool.tile([P, 512], bf16, tag="vres",
                                           name="vres")
                    evict(vtile[:], acc[st][:], on_act=(st % 2 == 1))
                    vres[half * 8 + st] = vtile

            # ============ Phase A: attention for one query group =============
            def phase_A(qg, psum_s, psum_c, psum_u, psum_b):
                nkb = 4 * qg + 4
                for h in range(HL):
                    e0 = h * HD
                    qt = [qres[(h * 2 + half, qg)] for half in range(2)]
                    ctxp = [psum_c.tile([P, 512], f32, tag="pc", name="pc")
                            for _ in range(2)]
                    sump = psum_u.tile([1, 512], f32, tag="pu", name="pu")
                    ps_list = [None] * nkb

                    def q0(kb):
                        # diagonal blocks only contribute to queries >= k
                        # block start; columns below that stay masked anyway
                        return max(0, (kb - 4 * qg) * P)

                    def emit_scores(kb, h=h, qt=qt, ps_list=ps_list):
                        c0_ = q0(kb)
                        ps = psum_s.tile([P, 512], f32, tag="ps", name="ps")
                        for half in range(2):
                            nc.tensor.matmul(
                                ps[:, c0_:],
                                kres[(h * 2 + half, kb // 4)][
                                    :, (kb % 4) * P:(kb % 4 + 1) * P],
                                qt[half][:, c0_:],
                                start=(half == 0), stop=(half == 1))
                        ps_list[kb] = ps

                    for kb0 in range(min(3, nkb)):
                        emit_scores(kb0)
                    for kb in range(nkb):
                        c0_ = q0(kb)
                        ps = ps_list[kb]
                        if kb >= 4 * qg:
                            # triangle mask on the leading 128 query columns
                            nc.vector.tensor_tensor(
                                ps[:, c0_:c0_ + P], ps[:, c0_:c0_ + P],
                                mask_sb[:, 0, 0:P], ADD)
                        p = pt_pool.tile([P, 512], bf16, tag="pt", name="pt")
                        nc.scalar.activation(p[:, c0_:], ps[:, c0_:], EXP)
                        if kb + 3 < nkb:
                            emit_scores(kb + 3)
                        nc.tensor.matmul(
                            sump[:, c0_:], ones_col[:], p[:, c0_:],
                            start=(kb == 0), stop=(kb == nkb - 1))
                        for half in range(2):
                            nc.tensor.matmul(
                                ctxp[half][:, c0_:],
                                vres[kb][:, e0 + half * P:e0 + (half + 1) * P],
                                p[:, c0_:],
                                start=(kb == 0), stop=(kb == nkb - 1))

                    # normalization: broadcast sum, reciprocal, scale
                    sum_sb = sums_pool.tile([1, 512], f32, tag="sums",
                                            name="sums")
                    nc.scalar.activation(sum_sb[:], sump[:], COPY)
                    bb = psum_b.tile([P, 512], f32, tag="pb", name="pb")
                    nc.tensor.matmul(bb[:], ones_row[:], sum_sb[:],
                                     start=True, stop=True)
                    rcp_bc = bbsb_pool.tile([P, 512], f32, tag="bbsb",
                                            name="bbsb")
                    nc.vector.reciprocal(rcp_bc[:], bb[:])
                    for half in range(2):
                        cst = cstg_pool.tile([P, 512], bf16, tag="cstg",
                                             name="cstg")
                        nc.vector.tensor_tensor(cst[:], ctxp[half][:],
                                                rcp_bc[:], MUL)
                        nc.gpsimd.dma_start(
                            out=t["ctxg"][qg][e0 + half * P:
                                              e0 + (half + 1) * P, :],
                            in_=cst[:])

                # ctx^T for this query group is complete -> AllGather it
                nc.gpsimd.collective_compute(
                    "AllGather",
                    mybir.AluOpType.bypass,
                    replica_groups=[list(range(NCORES))],
                    ins=[t["ctxg"][qg][:]],
                    outs=[t["ctxf"][qg][:]],
                )

            # ======== Phase O: out-projection rows [512qg, 512qg+512) ========
            cf_pre = {}   # qg -> prefetched leading cf tiles

            def prefetch_O(qg, n=8):
                tiles = []
                for dt in range(n):
                    ctile = cf_pool.tile([P, 512], bf16, tag="cf", name="cf")
                    nc.sync.dma_start(
                        out=ctile[:], in_=t["ctxf"][qg][dt * P:(dt + 1) * P, :])
                    tiles.append(ctile)
                cf_pre[qg] = tiles

            def phase_O(qg, psum_o):
                cf = list(cf_pre.pop(qg, []))
                for dt in range(len(cf), 32):
                    ctile = cf_pool.tile([P, 512], bf16, tag="cf", name="cf")
                    nc.sync.dma_start(
                        out=ctile[:], in_=t["ctxf"][qg][dt * P:(dt + 1) * P, :])
                    cf.append(ctile)
                po = [psum_o.tile([P, 512], f32, tag="po", name="po")
                      for _ in range(4)]
                for dt in range(32):
                    for st in range(4):
                        nc.tensor.matmul(
                            po[st][:],
                            cf[dt][:, st * P:(st + 1) * P],
                            wot[dt][:],
                            start=(dt == 0), stop=(dt == 31))
                for st in range(4):
                    stg = stage_pool.tile([P, 512], bf16, tag="stg",
                                          name="stg")
                    evict(stg[:], po[st][:], on_act=(st % 2 == 1))
                    nc.scalar.dma_start(
                        out=t["out"][qg * 512 + st * P:qg * 512 + (st + 1) * P, :],
                        in_=stg[:])

            # ============================ schedule ============================
            # hst half-1 tiles prefetch as soon as P0's V phase releases the
            # matching half-0 buffer (rings are free during late P0)
            hst1 = []

            def v_hook(dt):
                ht = hst_pool.tile([P, HALFW], bf16, tag="hst", name="hst")
                nc.sync.dma_start(
                    out=ht[:], in_=t["hsT"][dt * P:(dt + 1) * P, HALFW:S])
                hst1.append(ht)

            # tiny dummy AllGather: pays one-time collective setup early
            nc.gpsimd.collective_compute(
                "AllGather",
                mybir.AluOpType.bypass,
                replica_groups=[list(range(NCORES))],
                ins=[t["warm_in"][:]],
                outs=[t["warm_out"][:]],
            )

            with tc.tile_pool(name="psum_p0", bufs=8, space="PSUM") as psum_p:
                phase_P(0, hst0, psum_p, v_dt_hook=v_hook)

            # Wo^T column shard: needed first at O0, loads during A0/A1
            for dt in range(32):
                wtile = wot_pool.tile([P, EL], bf16, tag="wot", name="wot")
                nc.sync.dma_start(out=wtile[:],
                                  in_=t["woT2"][dt * P:(dt + 1) * P, :])
                wot.append(wtile)

            with contextlib.ExitStack() as stk_a:
                eca = stk_a.enter_context
                psum_s = eca(tc.tile_pool(name="ps_s", bufs=3, space="PSUM"))
                psum_c = eca(tc.tile_pool(name="ps_c", bufs=2, space="PSUM"))
                psum_u = eca(tc.tile_pool(name="ps_u", bufs=1, space="PSUM"))
                psum_b = eca(tc.tile_pool(name="ps_b", bufs=1, space="PSUM"))
                phase_A(0, psum_s, psum_c, psum_u, psum_b)
                phase_A(1, psum_s, psum_c, psum_u, psum_b)

            # O phases next: their cf loads read Shared scratch, which flows
            # even while an AllGather is in flight; P1's Local-DRAM weight
            # traffic would starve, so it waits until the collectives drain
            with tc.tile_pool(name="psum_o01", bufs=8, space="PSUM") as psum_o:
                phase_O(0, psum_o)
                phase_O(1, psum_o)

            with tc.tile_pool(name="psum_p1", bufs=8, space="PSUM") as psum_p:
                phase_P(1, hst1, psum_p)

            with contextlib.ExitStack() as stk_a:
                eca = stk_a.enter_context
                psum_s = eca(tc.tile_pool(name="ps_s2", bufs=3, space="PSUM"))
                psum_c = eca(tc.tile_pool(name="ps_c2", bufs=2, space="PSUM"))
                psum_u = eca(tc.tile_pool(name="ps_u2", bufs=1, space="PSUM"))
                psum_b = eca(tc.tile_pool(name="ps_b2", bufs=1, space="PSUM"))
                phase_A(2, psum_s, psum_c, psum_u, psum_b)
                phase_A(3, psum_s, psum_c, psum_u, psum_b)

            with tc.tile_pool(name="psum_o23", bufs=8, space="PSUM") as psum_o:
                phase_O(2, psum_o)
                phase_O(3, psum_o)


def _build():
    if "nc" in _CACHE:
        return _CACHE["nc"]
    nc = bacc.Bacc(None, num_devices=NCORES)
    t = {}
    t["hsT"] = nc.declare_dram_parameter("hsT", [D, S], bf16, isOutput=False)
    t["wqT"] = nc.declare_dram_parameter("wqT", [D, EL], bf16, isOutput=False)
    t["wkT"] = nc.declare_dram_parameter("wkT", [D, EL], bf16, isOutput=False)
    t["wvT"] = nc.declare_dram_parameter("wvT", [D, EL], bf16, isOutput=False)
    t["woT2"] = nc.declare_dram_parameter("woT2", [D, EL], bf16, isOutput=False)
    t["cosT"] = nc.declare_dram_parameter("cosT", [NROT2, S], f32, isOutput=False)
    t["sinT"] = nc.declare_dram_parameter("sinT", [NROT2, S], f32, isOutput=False)
    t["masks"] = nc.declare_dram_parameter("masks", [P, 4, 512], f32, isOutput=False)
    t["out"] = nc.declare_dram_parameter("out", [S, EL], bf16, isOutput=True)
    t["ctxg"] = [nc.dram_tensor(f"ctxg{qg}", [EL, 512], bf16)
                 for qg in range(4)]
    t["ctxf"] = [nc.dram_tensor(f"ctxf{qg}", [D, 512], bf16,
                                addr_space="Shared")
                 for qg in range(4)]
    t["warm_in"] = nc.dram_tensor("warm_in", [8, 16], bf16)
    t["warm_out"] = nc.dram_tensor("warm_out", [64, 16], bf16,
                                   addr_space="Shared")

    _emit(nc, t)
    nc.compile()
    _CACHE["nc"] = nc
    return nc


def _prep_inputs(hidden_states, Wq, Wk, Wv, Wo, attention_mask, position_ids):
    hs = np.asarray(hidden_states, np.float32).reshape(S, D)
    hsT = np.ascontiguousarray(hs.T).astype(ml_dtypes.bfloat16)

    pos = np.asarray(position_ids).reshape(S).astype(np.float32)
    inv = 10000.0 ** (-np.arange(0, ROT, 2, dtype=np.float32) / ROT)  # [32]
    ang = pos[:, None] * inv[None, :]                                  # [S, 32]
    cosT = np.ascontiguousarray(np.cos(ang).T).astype(np.float32)
    sinT = np.ascontiguousarray(np.sin(ang).T).astype(np.float32)

    am = np.asarray(attention_mask, np.float32).reshape(S, S)
    masks = np.empty((P, 4, 512), np.float32)
    for m in range(4):
        # transposed-score layout: mask[p, n] for k = m*128+p, q = n
        masks[:, m, :] = am[0:512, m * P:(m + 1) * P].T

    # within-head row permutation: even rot dims, odd rot dims, the rest
    perm1 = np.concatenate([np.arange(0, ROT, 2), np.arange(1, ROT, 2),
                            np.arange(ROT, HD)])
    perm = np.concatenate([perm1 + HD * j for j in range(HL)])

    Wq = np.asarray(Wq, np.float32)
    Wk = np.asarray(Wk, np.float32)
    Wv = np.asarray(Wv, np.float32)
    Wo = np.asarray(Wo, np.float32)
    scale = 1.0 / np.sqrt(np.float32(HD))

    in_maps = []
    for c in range(NCORES):
        rows = slice(c * EL, (c + 1) * EL)
        wq_c = Wq[rows][perm] * scale
        wk_c = Wk[rows][perm]
        wv_c = Wv[rows]
        in_maps.append({
            "hsT": hsT,
            "wqT": np.ascontiguousarray(wq_c.T).astype(ml_dtypes.bfloat16),
            "wkT": np.ascontiguousarray(wk_c.T).astype(ml_dtypes.bfloat16),
            "wvT": np.ascontiguousarray(wv_c.T).astype(ml_dtypes.bfloat16),
            "woT2": np.ascontiguousarray(Wo[rows, :].T).astype(ml_dtypes.bfloat16),
            "cosT": cosT,
            "sinT": sinT,
            "masks": masks,
        })
    return in_maps


def run(inputs, trace=False):
    """Run on HW. Returns (full_output, BassKernelResults)."""
    nc = _build()
    in_maps = _prep_inputs(**inputs)
    res = run_bass_kernel_spmd(nc, in_maps, list(range(NCORES)), trace=trace)
    # core c computed output columns [c*EL, (c+1)*EL)
    full = np.empty((S, D), np.float32)
    for c in range(NCORES):
        full[:, c * EL:(c + 1) * EL] = \
            np.asarray(res.results[c]["out"]).astype(np.float32)
    return full.reshape(B, S, D), res


def kernel(**inputs):
    full, _ = run(inputs, trace=False)
    return full


# revision 36
# speedup vs baseline: 1.1140x; 1.0022x over previous
"""GPT-J attention (B=1, S=2048, D=4096, H=16, HD=256, rot=64) on 8 TRN2 cores.

Strategy: tensor-parallel over heads (2 heads/core) for QKV+attention, then
column-parallel out-projection: after attention, each core AllGathers the full
ctx^T (its [512, 512] per query group -> shared [4096, 512]) and computes a
complete 512-column slice of the output (contracting the full d=4096), so no
ReduceScatter of 16.8MB partials is needed -- the host just concatenates the
8 column shards. Collective volume drops 8x and the reduction disappears.

Phases are interleaved so the AllGathers hide under PE work and the kernel
ends on matmuls, not comms:
  P0 (QKV for s<1024) -> A0,A1 (+AG0,AG1) -> O0,O1 -> P1 -> A2,A3 -> O2,O3

Host-side prep (cheap, numpy):
  - hsT = hidden_states.T so all device matmuls contract over the partition dim
  - per-core weight shards pre-transposed; Wq/Wk rows permuted within each head
    (even rot dims, odd rot dims, rest) so rotary becomes plain block ops
  - woT2 = Wo[cols of this core, :].T  (column shard of the out-projection)
  - 1/sqrt(HD) folded into Wq; sin/cos tables and causal mask tiles precomputed

Matmul operands are bf16 (fp32r measured 2 cyc/row on HW; bf16 is 1), all
accumulation in fp32 PSUM; softmax exp input, masks, rotary and the softmax
normalization run in fp32. q/K^T/V stay SBUF-resident between phases; ctx^T
roundtrips through DRAM via the AllGather. PSUM evictions alternate DVE/ACT.
"""

import contextlib
import numpy as np
import ml_dtypes

import concourse.bass as bass
import concourse.tile as tile
import concourse.mybir as mybir
from concourse import bacc
from concourse.bass_utils import run_bass_kernel_spmd

B, S, D = 1, 2048, 4096
H, HD, ROT = 16, 256, 64
NCORES = 8
HL = H // NCORES          # heads per core = 2
EL = D // NCORES          # local e width = 512
HALFW = S // 2            # 1024
P = 128
NROT2 = ROT // 2          # 32

f32 = mybir.dt.float32
bf16 = mybir.dt.bfloat16
EXP = mybir.ActivationFunctionType.Exp
COPY = mybir.ActivationFunctionType.Copy
ADD = mybir.AluOpType.add
MUL = mybir.AluOpType.mult
SUB = mybir.AluOpType.subtract

_CACHE = {}


def _emit(nc, t):
    """Emit the whole per-core program inside a TileContext."""
    with tile.TileContext(nc) as tc:
        with contextlib.ExitStack() as _stk:
            ec = _stk.enter_context
            const_pool = ec(tc.tile_pool(name="const", bufs=1))
            wpan_pool = ec(tc.tile_pool(name="wpan", bufs=4))
            stage_pool = ec(tc.tile_pool(name="stage", bufs=4))
            hst_pool = ec(tc.tile_pool(name="hst", bufs=32))
            qres_pool = ec(tc.tile_pool(name="qres", bufs=16))
            kres_pool = ec(tc.tile_pool(name="kres", bufs=16))
            vres_pool = ec(tc.tile_pool(name="vres", bufs=16))
            wot_pool = ec(tc.tile_pool(name="wot", bufs=32))
            cf_pool = ec(tc.tile_pool(name="cf", bufs=8))
            rot_pool = ec(tc.tile_pool(name="rot_scr", bufs=1))
            pt_pool = ec(tc.tile_pool(name="pt", bufs=4))
            bbsb_pool = ec(tc.tile_pool(name="bbsb", bufs=2))
            sums_pool = ec(tc.tile_pool(name="sums", bufs=1))
            cstg_pool = ec(tc.tile_pool(name="cstg", bufs=4))

            # first-half hsT loads go first so the first matmul starts ASAP
            hst0 = []
            for dt in range(32):
                ht = hst_pool.tile([P, HALFW], bf16, tag="hst", name="hst")
                nc.sync.dma_start(out=ht[:], in_=t["hsT"][dt * P:(dt + 1) * P,
                                                          0:HALFW])
                hst0.append(ht)

            ones_col = const_pool.tile([P, 1], bf16)
            nc.vector.memset(ones_col[:], 1.0)
            ones_row = const_pool.tile([1, P], f32)
            nc.vector.memset(ones_row[:], 1.0)
            cos_sb = const_pool.tile([NROT2, S], f32)
            nc.sync.dma_start(out=cos_sb[:], in_=t["cosT"][:])
            sin_sb = const_pool.tile([NROT2, S], f32)
            nc.sync.dma_start(out=sin_sb[:], in_=t["sinT"][:])
            mask_sb = const_pool.tile([P, 4, 512], f32)
            nc.sync.dma_start(out=mask_sb[:], in_=t["masks"][:])

            wot = []  # resident Wo^T column shard, loaded after P0 (below)

            qres = {}   # (et, scg) -> [128, 512] bf16  (q^T, rotary applied)
            kres = {}   # (et, scg) -> [128, 512] bf16  (k^T, rotary applied)
            vres = [None] * 16                      # [st16] -> [128s, 512e]

            def rot_evict(ps, stg, cols):
                # partitions [0:32) even pairs, [32:64) odd pairs, rest plain
                ca = cos_sb[:, cols:cols + 512]
                sa = sin_sb[:, cols:cols + 512]
                s1 = rot_pool.tile([NROT2, 512], f32, tag="rs1", name="rs1")
                s2 = rot_pool.tile([NROT2, 512], f32, tag="rs2", name="rs2")
                nc.vector.tensor_tensor(s1[:], ps[0:NROT2, :], ca, MUL)
                nc.vector.tensor_tensor(s2[:], ps[NROT2:ROT, :], sa, MUL)
                nc.vector.tensor_tensor(stg[0:NROT2, :], s1[:], s2[:], SUB)
                s3 = rot_pool.tile([NROT2, 512], f32, tag="rs1", name="rs1")
                s4 = rot_pool.tile([NROT2, 512], f32, tag="rs2", name="rs2")
                nc.vector.tensor_tensor(s3[:], ps[NROT2:ROT, :], ca, MUL)
                nc.vector.tensor_tensor(s4[:], ps[0:NROT2, :], sa, MUL)
                nc.vector.tensor_tensor(stg[NROT2:ROT, :], s3[:], s4[:], ADD)
                nc.scalar.activation(stg[ROT:P, :], ps[ROT:P, :], COPY)

            def evict(dst_ap, src_ps, on_act):
                if on_act:
                    nc.scalar.activation(dst_ap, src_ps, COPY)
                else:
                    nc.vector.tensor_copy(dst_ap, src_ps)

            # ================= Phase P: QKV projections for one half ==========
            def phase_P(half, hst, psum_p, parts=("q", "k", "v"),
                        v_dt_hook=None):
                c0 = half * HALFW
                # ---- Q then K: out layout [e, s], 2 waves of 4 banks ----
                for wname, is_q in (("wqT", True), ("wkT", False)):
                    if ("q" if is_q else "k") not in parts:
                        continue
                    acc = {}
                    for wave in range(2):
                        wacc = [psum_p.tile([P, 512], f32, tag="pp",
                                            name="pp") for _ in range(4)]
                        for dt in range(32):
                            wp = wpan_pool.tile([P, HD], bf16, tag="wp",
                                                name="wp")
                            nc.scalar.dma_start(
                                out=wp[:],
                                in_=t[wname][dt * P:(dt + 1) * P,
                                             wave * HD:(wave + 1) * HD])
                            for el in range(2):
                                for sc in range(2):
                                    nc.tensor.matmul(
                                        wacc[el * 2 + sc][:],
                                        wp[:, el * P:(el + 1) * P],
                                        hst[dt][:, sc * 512:(sc + 1) * 512],
                                        start=(dt == 0), stop=(dt == 31))
                        for el in range(2):
                            for sc in range(2):
                                acc[(wave * 2 + el) * 2 + sc] = wacc[el * 2 + sc]
                    for et in range(4):
                        for sc in range(2):
                            cols = c0 + sc * 512
                            scg = half * 2 + sc
                            ps = acc[et * 2 + sc]
                            dst = (qres_pool if is_q else kres_pool).tile(
                                [P, 512], bf16,
                                tag=("qres" if is_q else "kres"),
                                name=("qres" if is_q else "kres"))
                            if et in (0, 2):
                                rot_evict(ps, dst, cols)
                            else:
                                evict(dst[:], ps[:], on_act=(sc == 1))
                            (qres if is_q else kres)[(et, scg)] = dst

                # ---- V: natural layout [s, e] ----
                if "v" not in parts:
                    return
                acc = [psum_p.tile([P, 512], f32, tag="pp", name="pp")
                       for _ in range(8)]
                for dt in range(32):
                    wp = wpan_pool.tile([P, EL], bf16, tag="wp", name="wp")
                    nc.scalar.dma_start(
                        out=wp[:], in_=t["wvT"][dt * P:(dt + 1) * P, :])
                    for st in range(8):
                        nc.tensor.matmul(
                            acc[st][:],
                            hst[dt][:, st * P:(st + 1) * P],
                            wp[:],
                            start=(dt == 0), stop=(dt == 31))
                    if v_dt_hook is not None:
                        v_dt_hook(dt)
                for st in range(8):
                    vtile = vres_pool.tile([P, 512], bf16, tag="vres",
                                           name="vres")
                    evict(vtile[:], acc[st][:], on_act=(st % 2 == 1))
                    vres[half * 8 + st] = vtile

            # ============ Phase A: attention for one query group =============
            def phase_A(qg, psum_s, psum_c, psum_u, psum_b):
                nkb = 4 * qg + 4
                for h in range(HL):
                    e0 = h * HD
                    qt = [qres[(h * 2 + half, qg)] for half in range(2)]
                    ctxp = [psum_c.tile([P, 512], f32, tag="pc", name="pc")
                            for _ in range(2)]
                    sump = psum_u.tile([1, 512], f32, tag="pu", name="pu")
                    ps_list = [None] * nkb

                    def q0(kb):
                        # diagonal blocks only contribute to queries >= k
                        # block start; columns below that stay masked anyway
                        return max(0, (kb - 4 * qg) * P)

                    def emit_scores(kb, h=h, qt=qt, ps_list=ps_list):
                        c0_ = q0(kb)
                        ps = psum_s.tile([P, 512], f32, tag="ps", name="ps")
                        for half in range(2):
                            nc.tensor.matmul(
                                ps[:, c0_:],
                                kres[(h * 2 + half, kb // 4)][
                                    :, (kb % 4) * P:(kb % 4 + 1) * P],
                                qt[half][:, c0_:],
                                start=(half == 0), stop=(half == 1))
                        ps_list[kb] = ps

                    for kb0 in range(min(3, nkb)):
                        emit_scores(kb0)
                    for kb in range(nkb):
                        c0_ = q0(kb)
                        ps = ps_list[kb]
                        if kb >= 4 * qg:
                            # triangle mask on the leading 128 query columns
                            nc.vector.tensor_tensor(
                                ps[:, c0_:c0_ + P], ps[:, c0_:c0_ + P],
                                mask_sb[:, 0, 0:P], ADD)
                        p = pt_pool.tile([P, 512], bf16, tag="pt", name="pt")
                        nc.scalar.activation(p[:, c0_:], ps[:, c0_:], EXP)
                        if kb + 3 < nkb:
                            emit_scores(kb + 3)
                        nc.tensor.matmul(
                            sump[:, c0_:], ones_col[:], p[:, c0_:],
                            start=(kb == 0), stop=(kb == nkb - 1))
                        for half in range(2):
                            nc.tensor.matmul(
                                ctxp[half][:, c0_:],
                                vres[kb][:, e0 + half * P:e0 + (half + 1) * P],
                                p[:, c0_:],
                                start=(kb == 0), stop=(kb == nkb - 1))

                    # normalization: broadcast sum, reciprocal, scale
                    sum_sb = sums_pool.tile([1, 512], f32, tag="sums",
                                            name="sums")
                    nc.scalar.activation(sum_sb[:], sump[:], COPY)
                    bb = psum_b.tile([P, 512], f32, tag="pb", name="pb")
                    nc.tensor.matmul(bb[:], ones_row[:], sum_sb[:],
                                     start=True, stop=True)
                    rcp_bc = bbsb_pool.tile([P, 512], f32, tag="bbsb",
                                            name="bbsb")
                    nc.vector.reciprocal(rcp_bc[:], bb[:])
                    for half in range(2):
                        cst = cstg_pool.tile([P, 512], bf16, tag="cstg",
                                             name="cstg")
                        nc.vector.tensor_tensor(cst[:], ctxp[half][:],
                                                rcp_bc[:], MUL)
                        nc.gpsimd.dma_start(
                            out=t["ctxg"][qg][e0 + half * P:
                                              e0 + (half + 1) * P, :],
                            in_=cst[:])

                # ctx^T for this query group is complete -> AllGather it
                nc.gpsimd.collective_compute(
                    "AllGather",
                    mybir.AluOpType.bypass,
                    replica_groups=[list(range(NCORES))],
                    ins=[t["ctxg"][qg][:]],
                    outs=[t["ctxf"][qg][:]],
                )

            # ======== Phase O: out-projection rows [512qg, 512qg+512) ========
            cf_pre = {}   # qg -> prefetched leading cf tiles

            def prefetch_O(qg, n=8):
                tiles = []
                for dt in range(n):
                    ctile = cf_pool.tile([P, 512], bf16, tag="cf", name="cf")
                    nc.sync.dma_start(
                        out=ctile[:], in_=t["ctxf"][qg][dt * P:(dt + 1) * P, :])
                    tiles.append(ctile)
                cf_pre[qg] = tiles

            def phase_O(qg, psum_o):
                cf = list(cf_pre.pop(qg, []))
                for dt in range(len(cf), 32):
                    ctile = cf_pool.tile([P, 512], bf16, tag="cf", name="cf")
                    nc.sync.dma_start(
                        out=ctile[:], in_=t["ctxf"][qg][dt * P:(dt + 1) * P, :])
                    cf.append(ctile)
                po = [psum_o.tile([P, 512], f32, tag="po", name="po")
                      for _ in range(4)]
                for dt in range(32):
                    for st in range(4):
                        nc.tensor.matmul(
                            po[st][:],
                            cf[dt][:, st * P:(st + 1) * P],
                            wot[dt][:],
                            start=(dt == 0), stop=(dt == 31))
                for st in range(4):
                    stg = stage_pool.tile([P, 512], bf16, tag="stg",
                                          name="stg")
                    evict(stg[:], po[st][:], on_act=(st % 2 == 1))
                    nc.scalar.dma_start(
                        out=t["out"][qg * 512 + st * P:qg * 512 + (st + 1) * P, :],
                        in_=stg[:])

            # ============================ schedule ============================
            # hst half-1 tiles prefetch as soon as P0's V phase releases the
            # matching half-0 buffer (rings are free during late P0)
            hst1 = []

            def v_hook(dt):
                ht = hst_pool.tile([P, HALFW], bf16, tag="hst", name="hst")
                nc.sync.dma_start(
                    out=ht[:], in_=t["hsT"][dt * P:(dt + 1) * P, HALFW:S])
                hst1.append(ht)

            # tiny dummy AllGather: pays one-time collective setup early
            nc.gpsimd.collective_compute(
                "AllGather",
                mybir.AluOpType.bypass,
                replica_groups=[list(range(NCORES))],
                ins=[t["warm_in"][:]],
                outs=[t["warm_out"][:]],
            )

            with tc.tile_pool(name="psum_p0", bufs=8, space="PSUM") as psum_p:
                phase_P(0, hst0, psum_p, v_dt_hook=v_hook)

            # Wo^T column shard: needed first at O0, loads during A0/A1
            for dt in range(32):
                wtile = wot_pool.tile([P, EL], bf16, tag="wot", name="wot")
                nc.sync.dma_start(out=wtile[:],
                                  in_=t["woT2"][dt * P:(dt + 1) * P, :])
                wot.append(wtile)

            with contextlib.ExitStack() as stk_a:
                eca = stk_a.enter_context
                psum_s = eca(tc.tile_pool(name="ps_s", bufs=4, space="PSUM"))
                psum_c = eca(tc.tile_pool(name="ps_c", bufs=2, space="PSUM"))
                psum_u = eca(tc.tile_pool(name="ps_u", bufs=1, space="PSUM"))
                psum_b = eca(tc.tile_pool(name="ps_b", bufs=1, space="PSUM"))
                phase_A(0, psum_s, psum_c, psum_u, psum_b)
                phase_A(1, psum_s, psum_c, psum_u, psum_b)

            # O phases next: their cf loads read Shared scratch, which flows
            # even while an AllGather is in flight; P1's Local-DRAM weight
            # traffic would starve, so it waits until the collectives drain
            with tc.tile_pool(name="psum_o01", bufs=8, space="PSUM") as psum_o:
                phase_O(0, psum_o)
                phase_O(1, psum_o)

            with tc.tile_pool(name="psum_p1", bufs=8, space="PSUM") as psum_p:
                phase_P(1, hst1, psum_p)

            with contextlib.ExitStack() as stk_a:
                eca = stk_a.enter_context
                psum_s = eca(tc.tile_pool(name="ps_s2", bufs=4, space="PSUM"))
                psum_c = eca(tc.tile_pool(name="ps_c2", bufs=2, space="PSUM"))
                psum_u = eca(tc.tile_pool(name="ps_u2", bufs=1, space="PSUM"))
                psum_b = eca(tc.tile_pool(name="ps_b2", bufs=1, space="PSUM"))
                phase_A(2, psum_s, psum_c, psum_u, psum_b)
                phase_A(3, psum_s, psum_c, psum_u, psum_b)

            with tc.tile_pool(name="psum_o23", bufs=8, space="PSUM") as psum_o:
                phase_O(2, psum_o)
                phase_O(3, psum_o)


def _build():
    if "nc" in _CACHE:
        return _CACHE["nc"]
    nc = bacc.Bacc(None, num_devices=NCORES)
    t = {}
    t["hsT"] = nc.declare_dram_parameter("hsT", [D, S], bf16, isOutput=False)
    t["wqT"] = nc.declare_dram_parameter("wqT", [D, EL], bf16, isOutput=False)
    t["wkT"] = nc.declare_dram_parameter("wkT", [D, EL], bf16, isOutput=False)
    t["wvT"] = nc.declare_dram_parameter("wvT", [D, EL], bf16, isOutput=False)
    t["woT2"] = nc.declare_dram_parameter("woT2", [D, EL], bf16, isOutput=False)
    t["cosT"] = nc.declare_dram_parameter("cosT", [NROT2, S], f32, isOutput=False)
    t["sinT"] = nc.declare_dram_parameter("sinT", [NROT2, S], f32, isOutput=False)
    t["masks"] = nc.declare_dram_parameter("masks", [P, 4, 512], f32, isOutput=False)
    t["out"] = nc.declare_dram_parameter("out", [S, EL], bf16, isOutput=True)
    t["ctxg"] = [nc.dram_tensor(f"ctxg{qg}", [EL, 512], bf16)
                 for qg in range(4)]
    t["ctxf"] = [nc.dram_tensor(f"ctxf{qg}", [D, 512], bf16,
                                addr_space="Shared")
                 for qg in range(4)]
    t["warm_in"] = nc.dram_tensor("warm_in", [8, 16], bf16)
    t["warm_out"] = nc.dram_tensor("warm_out", [64, 16], bf16,
                                   addr_space="Shared")

    _emit(nc, t)
    nc.compile()
    _CACHE["nc"] = nc
    return nc


def _prep_inputs(hidden_states, Wq, Wk, Wv, Wo, attention_mask, position_ids):
    hs = np.asarray(hidden_states, np.float32).reshape(S, D)
    hsT = np.ascontiguousarray(hs.T).astype(ml_dtypes.bfloat16)

    pos = np.asarray(position_ids).reshape(S).astype(np.float32)
    inv = 10000.0 ** (-np.arange(0, ROT, 2, dtype=np.float32) / ROT)  # [32]
    ang = pos[:, None] * inv[None, :]                                  # [S, 32]
    cosT = np.ascontiguousarray(np.cos(ang).T).astype(np.float32)
    sinT = np.ascontiguousarray(np.sin(ang).T).astype(np.float32)

    am = np.asarray(attention_mask, np.float32).reshape(S, S)
    masks = np.empty((P, 4, 512), np.float32)
    for m in range(4):
        # transposed-score layout: mask[p, n] for k = m*128+p, q = n
        masks[:, m, :] = am[0:512, m * P:(m + 1) * P].T

    # within-head row permutation: even rot dims, odd rot dims, the rest
    perm1 = np.concatenate([np.arange(0, ROT, 2), np.arange(1, ROT, 2),
                            np.arange(ROT, HD)])
    perm = np.concatenate([perm1 + HD * j for j in range(HL)])

    Wq = np.asarray(Wq, np.float32)
    Wk = np.asarray(Wk, np.float32)
    Wv = np.asarray(Wv, np.float32)
    Wo = np.asarray(Wo, np.float32)
    scale = 1.0 / np.sqrt(np.float32(HD))

    in_maps = []
    for c in range(NCORES):
        rows = slice(c * EL, (c + 1) * EL)
        wq_c = Wq[rows][perm] * scale
        wk_c = Wk[rows][perm]
        wv_c = Wv[rows]
        in_maps.append({
            "hsT": hsT,
            "wqT": np.ascontiguousarray(wq_c.T).astype(ml_dtypes.bfloat16),
            "wkT": np.ascontiguousarray(wk_c.T).astype(ml_dtypes.bfloat16),
            "wvT": np.ascontiguousarray(wv_c.T).astype(ml_dtypes.bfloat16),
            "woT2": np.ascontiguousarray(Wo[rows, :].T).astype(ml_dtypes.bfloat16),
            "cosT": cosT,
            "sinT": sinT,
            "masks": masks,
        })
    return in_maps


def run(inputs, trace=False):
    """Run on HW. Returns (full_output, BassKernelResults)."""
    nc = _build()
    in_maps = _prep_inputs(**inputs)
    res = run_bass_kernel_spmd(nc, in_maps, list(range(NCORES)), trace=trace)
    # core c computed output columns [c*EL, (c+1)*EL)
    full = np.empty((S, D), np.float32)
    for c in range(NCORES):
        full[:, c * EL:(c + 1) * EL] = \
            np.asarray(res.results[c]["out"]).astype(np.float32)
    return full.reshape(B, S, D), res


def kernel(**inputs):
    full, _ = run(inputs, trace=False)
    return full
